# revision 38
# baseline (speedup 1.0000x reference)
"""Trainium2 Bass kernel for the blended-MoE actor network.

Math: reference computes, per sample,
    g1 = relu(bw1 @ s + bb1); g2 = relu(bw2 @ g1 + bb2)
    c  = softmax(bwo @ g2 + bbo)            # 2 experts
    h1 = relu(blend(W1_e, s)); h2 = relu(blend(W2_e, h1))
    mu = tanh(blend(Wm_e, h2))
with blend(W_e, x) = sum_e c_e (W_e x + b_e).

Since NE=2 and c0+c1=1:  c0 = sigmoid((bwo[0]-bwo[1]) @ g2 + dbo)  and
    blend(W_e, x) = W_1 x + b_1 + c0 * (dW x + db),  dW = W_0-W_1.
The c0 * (dW x) term is computed by scaling the matmul INPUT per-sample
(x_c = C0 .* x) so both expert contributions accumulate into one PSUM
group. Rank-1 bias terms c0*db ride along: an appended ones-row on the
states makes row 376 of the scaled states equal c0 (feeding the db
column of the augmented diff weights), and for later layers K=1 matmuls
against the c0 row add c0*db directly.

Layout: activations are [features, batch] on-chip (host pre-transposes
states and appends a ones row); batch tiles of N=512 (one PSUM bank per
matmul). The router logit-diff matmul uses wd replicated across 128
output columns so its PSUM output holds the logit diff in every
partition row: one sigmoid yields the broadcast C0 tile for free.

The per-tile dataflow is a serial chain (blend MLP -> C0 -> scaled
inputs -> expert layers), so instructions are emitted in an explicit
software-pipelined order across batch tiles -- the TensorEngine stream
interleaves expert layers of tiles t, t-1, t-2 with the blend MLP of
tile t+2, keeping the PE free of cross-engine round-trip stalls.

Sharding: pure data parallel over 8 cores (batch 65536 -> 8 x 8192).
"""

import ml_dtypes
import numpy as np

import concourse.bass as bass
import concourse.mybir as mybir
import concourse.tile as tile
from concourse import bacc
from concourse.bass_utils import run_bass_kernel_spmd

N_CORES = 8
B = 65536
BS = B // N_CORES  # 8192 per core
NI = 376  # state features
NIA = NI + 1  # + ones row
NA = 17  # actions
BH = 128  # blending hidden
NT = 512  # batch tile (matmul free dim, one PSUM bank)
T = BS // NT  # 16 tiles per core

F32 = mybir.dt.float32
# bf16 compute: matmul streams 1 col/cycle warm (f32r measures 2), DVE
# tensor_tensor gets 2x mode, DMA bytes halve. rel err ~8e-3 << 2e-2.
DT = mybir.dt.bfloat16
DT_NP = ml_dtypes.bfloat16

AF = mybir.ActivationFunctionType
KCH = ((0, 128), (128, 256), (256, NIA))  # K chunks of the state dim


# ---------------------------------------------------------------- weights
# All stationary operands are packed into one [128, WCOLS] host array;
# each lhsT is a column slice [0:K, off:off+M]. Rows >= K are zero.


class _Pack:
    def __init__(self):
        self.cols = []
        self.off = 0

    def add(self, arr):  # arr [K, M] -> returns (off, K, M)
        k, m = arr.shape
        assert k <= 128
        a = np.zeros((128, m), np.float32)
        a[:k] = arr
        off = self.off
        self.cols.append(a)
        self.off += m
        return (off, k, m)

    def data(self):
        return np.concatenate(self.cols, axis=1)


def _prep_weights(p, bw1, bb1, bw2, bb2, bwo, bbo, ew1, eb1, ew2, eb2, ewm, ebm):
    d = {}
    # blend L1: lhsT [377, 128] (= [bw1.T; bb1]) in 3 K-chunks
    w1a = np.concatenate([bw1.T, bb1[None, :]], axis=0)
    d["bl1"] = [p.add(w1a[k0:k1]) for k0, k1 in KCH]
    d["bl2"] = [p.add(bw2.T)]
    d["bb2"] = p.add(bb2[:, None])
    # router logit diff, replicated to 128 output columns
    wd = (bwo[0] - bwo[1])[:, None]
    d["wd"] = [p.add(np.repeat(wd, 128, axis=1))]
    d["bd"] = p.add(np.full((128, 1), bbo[0] - bbo[1], np.float32))
    # expert L1: base = expert1, diff = expert0 - expert1; bias rows appended
    e1b = np.concatenate([ew1[1].T, eb1[1][None, :]], axis=0)  # [377, 256]
    e1d = np.concatenate([(ew1[0] - ew1[1]).T, (eb1[0] - eb1[1])[None, :]], axis=0)
    d["e1b"] = [[p.add(e1b[k0:k1, m : m + 128]) for k0, k1 in KCH] for m in (0, 128)]
    d["e1d"] = [[p.add(e1d[k0:k1, m : m + 128]) for k0, k1 in KCH] for m in (0, 128)]
    # expert L2
    e2b = ew2[1].T  # [256, 128]
    e2d = (ew2[0] - ew2[1]).T
    d["e2b"] = [p.add(e2b[0:128]), p.add(e2b[128:256])]
    d["e2d"] = [p.add(e2d[0:128]), p.add(e2d[128:256])]
    d["db2"] = [p.add((eb2[0] - eb2[1])[None, :])]  # [1, 128]
    d["b2"] = p.add(eb2[1][:, None])
    # expert out
    d["emb"] = [p.add(ewm[1].T)]  # [128, 17]
    d["emd"] = [p.add((ewm[0] - ewm[1]).T)]
    d["dbm"] = [p.add((ebm[0] - ebm[1])[None, :])]  # [1, 17]
    d["bm"] = p.add(ebm[1][:, None])
    return d


# ---------------------------------------------------------------- kernel


def _build(wd, wcols):
    nc = bacc.Bacc("TRN2", target_bir_lowering=False, debug=False,
                   num_devices=N_CORES)
    xs = nc.declare_dram_parameter("xs", [NIA, BS], DT, isOutput=False)
    wk = nc.declare_dram_parameter("wk", [128, wcols], DT, isOutput=False)
    out = nc.declare_dram_parameter("out", [NA, BS], F32, isOutput=True)

    with tile.TileContext(nc) as tc:
        with (
            tc.tile_pool(name="wpool", bufs=1) as wpool,
            tc.tile_pool(name="spool", bufs=3) as spool,
            tc.tile_pool(name="scpool", bufs=4) as scpool,
            tc.tile_pool(name="gpool", bufs=3) as gpool,
            tc.tile_pool(name="cpool", bufs=4) as cpool,
            tc.tile_pool(name="hpool", bufs=3) as hpool,
            tc.tile_pool(name="opool", bufs=2) as opool,
            tc.tile_pool(name="psum", bufs=1, space="PSUM") as pp,
        ):
            # weight pack arrives as two separate tiles: the blend weights
            # (packed first) unblock the first matmuls early, and the split
            # avoids a false whole-tile dependency on the second DMA
            wsplit = wd["e1b"][0][0][0]  # end of the blend-weight section
            wkt1 = wpool.tile([128, wsplit], DT)
            nc.sync.dma_start(wkt1[:], wk[:, 0:wsplit])
            wkt2 = wpool.tile([128, wcols - wsplit], DT)

            def W(desc):
                off, k, m = desc
                if off + m <= wsplit:
                    return wkt1[0:k, off : off + m]
                return wkt2[0:k, off - wsplit : off - wsplit + m]

            # per-tile (t) and per-pair (p = t//2) live tensors; pairs are
            # 1024 wide so DMA and GpSimd muls run at half the instruction
            # and semaphore count
            s = {}
            sc = {}
            c0 = {}
            g1 = {}
            g2 = {}
            h1 = {}
            h1c = {}
            h2 = {}
            h2c = {}
            mu = {}

            def csl(t):
                return slice(t * NT, (t + 1) * NT)

            def psl(t):  # slice of tile t within its pair tile
                return slice((t % 2) * NT, (t % 2 + 1) * NT)

            def sv(t, ci):  # state chunk view for tile t
                return s[t // 2][ci][:, psl(t)]

            def dma_in(p, chunks=(0, 1, 2)):
                # pair p: columns [p*2NT, (p+1)*2NT); chunks may be staggered
                # across iterations to spread SBUF write-port contention
                if p not in s:
                    s[p] = [None, None, None]
                for ci in chunks:
                    k0, k1 = KCH[ci]
                    st = spool.tile([k1 - k0, 2 * NT], DT, tag=f"s{ci}",
                                    name=f"s{ci}_{p}", bufs=4)
                    nc.sync.dma_start(st[:], xs[k0:k1, p * 2 * NT : (p + 1) * 2 * NT])
                    s[p][ci] = st

            def blend_g1(t):
                pg1 = pp.tile([BH, NT], F32, tag="g1", name=f"pg1_{t}", bufs=2)
                for ci in range(3):
                    nc.tensor.matmul(pg1[:], W(wd["bl1"][ci]), sv(t, ci),
                                     start=(ci == 0), stop=(ci == 2))
                g1[t] = gpool.tile([BH, NT], DT, tag="g1", name=f"g1_{t}")
                nc.vector.tensor_scalar_max(g1[t][:], pg1[:], 0.0)

            def blend_g2(t):
                pg2 = pp.tile([BH, NT], F32, tag="g2", name=f"pg2_{t}")
                nc.tensor.matmul(pg2[:], W(wd["bl2"][0]), g1[t][:],
                                 start=True, stop=True)
                g2[t] = gpool.tile([BH, NT], DT, tag="g2", name=f"g2_{t}")
                nc.scalar.activation(g2[t][:], pg2[:], AF.Relu, bias=W(wd["bb2"]))

            def blend_d(t):
                pd = pp.tile([128, NT], F32, tag="d", name=f"pd_{t}")
                nc.tensor.matmul(pd[:], W(wd["wd"][0]), g2[t][:],
                                 start=True, stop=True)
                p = t // 2
                if t % 2 == 0:
                    c0[p] = cpool.tile([128, 2 * NT], DT, tag="c0", name=f"c0_{p}")
                nc.scalar.activation(c0[p][:, psl(t)], pd[:], AF.Sigmoid,
                                     bias=W(wd["bd"]))
                # scaled states for tile t; chunk 0 on DVE, chunks 1/2 on
                # GpSimd (SBUF-only). Row 120 of chunk 2 becomes c0 via the
                # ones-row of xs.
                sc[t] = []
                for ci, (k0, k1) in enumerate(KCH):
                    tt = scpool.tile([k1 - k0, NT], DT, tag=f"sc{ci}",
                                     name=f"sc{ci}_{t}")
                    eng = nc.vector if ci == 0 else nc.gpsimd
                    eng.tensor_mul(tt[:], sv(t, ci), c0[p][0 : k1 - k0, psl(t)])
                    sc[t].append(tt)

            def exp_l1(t):
                h1[t] = []
                h1c[t] = []
                for m in range(2):
                    ph = pp.tile([128, NT], F32, tag=f"h1{m}", name=f"ph1{m}_{t}")
                    for ci in range(3):
                        nc.tensor.matmul(ph[:], W(wd["e1b"][m][ci]), sv(t, ci),
                                         start=(ci == 0), stop=False)
                    for ci in range(3):
                        nc.tensor.matmul(ph[:], W(wd["e1d"][m][ci]), sc[t][ci][:],
                                         start=False, stop=(ci == 2))
                    ht = hpool.tile([128, NT], DT, tag=f"h1{m}", name=f"h1{m}_{t}")
                    nc.vector.tensor_scalar_max(ht[:], ph[:], 0.0)
                    h1[t].append(ht)
                    htc = hpool.tile([128, NT], DT, tag=f"h1c{m}",
                                     name=f"h1c{m}_{t}")
                    nc.vector.tensor_mul(htc[:], ht[:], c0[t // 2][:, psl(t)])
                    h1c[t].append(htc)

            ph2s = {}
            pmus = {}

            def exp_l2_mm(t):
                ph2s[t] = pp.tile([128, NT], F32, tag="h2", name=f"ph2_{t}")
                ph2 = ph2s[t]
                nc.tensor.matmul(ph2[:], W(wd["e2b"][0]), h1[t][0][:],
                                 start=True, stop=False)
                nc.tensor.matmul(ph2[:], W(wd["e2b"][1]), h1[t][1][:],
                                 start=False, stop=False)
                nc.tensor.matmul(ph2[:], W(wd["e2d"][0]), h1c[t][0][:],
                                 start=False, stop=False)
                nc.tensor.matmul(ph2[:], W(wd["e2d"][1]), h1c[t][1][:],
                                 start=False, stop=False)

            def exp_l3_mm(t):
                pmus[t] = pp.tile([NA, NT], F32, tag="mu", name=f"pmu_{t}")
                pmu = pmus[t]
                nc.tensor.matmul(pmu[:], W(wd["emb"][0]), h2[t][:],
                                 start=True, stop=False)
                nc.tensor.matmul(pmu[:], W(wd["emd"][0]), h2c[t][:],
                                 start=False, stop=False)

            def bias_mms(t2, t3):
                # two K=1 rank-1 bias matmuls; db2 occupies PE row group 0,
                # dbm row group 1 (weights at partition 32), so adjacent
                # instructions stream concurrently through the array
                if t2 is not None:
                    nc.tensor.matmul(ph2s[t2][:], W(wd["db2"][0]),
                                     c0[t2 // 2][0:1, psl(t2)],
                                     start=False, stop=True)
                if t3 is not None:
                    nc.tensor.matmul(pmus[t3][:], W(wd["dbm"][0]),
                                     c0[t3 // 2][0:1, psl(t3)],
                                     start=False, stop=True)

            def exp_l2_post(t):
                ph2 = ph2s.pop(t)
                h2[t] = hpool.tile([128, NT], DT, tag="h2", name=f"h2_{t}")
                nc.scalar.activation(h2[t][:], ph2[:], AF.Relu, bias=W(wd["b2"]))
                h2c[t] = hpool.tile([128, NT], DT, tag="h2c", name=f"h2c_{t}")
                nc.vector.tensor_mul(h2c[t][:], h2[t][:], c0[t // 2][:, psl(t)])
                del g1[t], g2[t], sc[t]
                if t % 2 == 1:
                    del s[t // 2]

            def exp_l3_post(t):
                pmu = pmus.pop(t)
                p = t // 2
                if t % 2 == 0:
                    mu[p] = opool.tile([NA, 2 * NT], F32, tag="mu", name=f"mu_{p}")
                nc.scalar.activation(mu[p][:, psl(t)], pmu[:], AF.Tanh,
                                     bias=W(wd["bm"]))
                if t >= T - 2:
                    nc.sync.dma_start(out[:, csl(t)], mu[p][:, psl(t)])
                    if t % 2 == 1:
                        del mu[p], c0[p]
                elif t % 2 == 1:
                    nc.sync.dma_start(out[:, p * 2 * NT : (p + 1) * 2 * NT],
                                      mu[p][:])
                    del mu[p], c0[p]
                del h1[t], h1c[t], h2[t], h2c[t]

            def exp_l2(t):
                exp_l2_mm(t)
                bias_mms(t, None)
                exp_l2_post(t)

            def exp_l3(t):
                exp_l3_mm(t)
                bias_mms(None, t)
                exp_l3_post(t)

            # -------- software-pipelined emission --------
            # prologue: state pairs 0..1, blend chains for tiles 0 and 1
            # (interleaved to overlap their serial MM->ACT/DVE hops)
            dma_in(0)
            dma_in(1)
            nc.sync.dma_start(wkt2[:], wk[:, wsplit:])
            dma_in(2, (0,))
            blend_g1(0)
            blend_g1(1)
            blend_g2(0)
            blend_g2(1)
            blend_d(0)
            blend_d(1)
            # steady state: iteration t runs L1(t), L2(t-1), L3(t-2) and
            # the blend MLP of t+2 spliced between expert blocks
            for t in range(T):
                if t % 2 == 1 and (t + 5) // 2 < T // 2:
                    dma_in((t + 5) // 2, (0,))
                if t % 2 == 0 and 2 <= (t + 4) // 2 < T // 2:
                    dma_in((t + 4) // 2, (1, 2))
                if t + 2 < T:
                    blend_g1(t + 2)
                exp_l1(t)
                if t + 2 < T:
                    blend_g2(t + 2)
                if t >= 2:
                    exp_l3_mm(t - 2)
                if t >= 1:
                    exp_l2_mm(t - 1)
                bias_mms(t - 1 if t >= 1 else None, t - 2 if t >= 2 else None)
                if t + 2 < T:
                    blend_d(t + 2)
                if t >= 1:
                    exp_l2_post(t - 1)
                if t >= 2:
                    exp_l3_post(t - 2)
            exp_l2(T - 1)
            exp_l3(T - 2)
            exp_l3(T - 1)
    nc.finalize()
    return nc


_CACHE = {}


def kernel(**inputs) -> np.ndarray:
    states = np.asarray(inputs["states"], np.float32)
    pack = _Pack()
    wdesc = _prep_weights(
        pack,
        *[
            np.asarray(inputs[k], np.float32)
            for k in ("bw1", "bb1", "bw2", "bb2", "bwo", "bbo",
                      "ew1", "eb1", "ew2", "eb2", "ewm", "ebm")
        ],
    )
    wdata = pack.data().astype(DT_NP)  # [128, wcols]

    if "nc" not in _CACHE:
        _CACHE["nc"] = _build(wdesc, wdata.shape[1])
    nc = _CACHE["nc"]

    in_maps = []
    for c in range(N_CORES):
        shard = states[c * BS : (c + 1) * BS]  # [BS, NI]
        xs = np.empty((NIA, BS), np.float32)
        xs[:NI] = shard.T
        xs[NI] = 1.0
        in_maps.append({"xs": xs.astype(DT_NP), "wk": wdata})

    res = run_bass_kernel_spmd(nc, in_maps, core_ids=list(range(N_CORES)))
    out = np.empty((B, NA), np.float32)
    for c in range(N_CORES):
        out[c * BS : (c + 1) * BS] = res.results[c]["out"].T
    return out


# revision 39
# speedup vs baseline: 1.0210x; 1.0210x over previous
"""Trainium2 Bass kernel for the blended-MoE actor network.

Math: reference computes, per sample,
    g1 = relu(bw1 @ s + bb1); g2 = relu(bw2 @ g1 + bb2)
    c  = softmax(bwo @ g2 + bbo)            # 2 experts
    h1 = relu(blend(W1_e, s)); h2 = relu(blend(W2_e, h1))
    mu = tanh(blend(Wm_e, h2))
with blend(W_e, x) = sum_e c_e (W_e x + b_e).

Since NE=2 and c0+c1=1:  c0 = sigmoid((bwo[0]-bwo[1]) @ g2 + dbo)  and
    blend(W_e, x) = W_1 x + b_1 + c0 * (dW x + db),  dW = W_0-W_1.
The c0 * (dW x) term is computed by scaling the matmul INPUT per-sample
(x_c = C0 .* x) so both expert contributions accumulate into one PSUM
group. Rank-1 bias terms c0*db ride along: an appended ones-row on the
states makes row 376 of the scaled states equal c0 (feeding the db
column of the augmented diff weights), and for later layers K=1 matmuls
against the c0 row add c0*db directly.

Layout: activations are [features, batch] on-chip (host pre-transposes
states and appends a ones row); batch tiles of N=512 (one PSUM bank per
matmul). The router logit-diff matmul uses wd replicated across 128
output columns so its PSUM output holds the logit diff in every
partition row: one sigmoid yields the broadcast C0 tile for free.

The per-tile dataflow is a serial chain (blend MLP -> C0 -> scaled
inputs -> expert layers), so instructions are emitted in an explicit
software-pipelined order across batch tiles -- the TensorEngine stream
interleaves expert layers of tiles t, t-1, t-2 with the blend MLP of
tile t+2, keeping the PE free of cross-engine round-trip stalls.

Sharding: pure data parallel over 8 cores (batch 65536 -> 8 x 8192).
"""

import ml_dtypes
import numpy as np

import concourse.bass as bass
import concourse.mybir as mybir
import concourse.tile as tile
from concourse import bacc
from concourse.bass_utils import run_bass_kernel_spmd

N_CORES = 8
B = 65536
BS = B // N_CORES  # 8192 per core
NI = 376  # state features
NIA = NI + 1  # + ones row
NA = 17  # actions
BH = 128  # blending hidden
NT = 512  # batch tile (matmul free dim, one PSUM bank)
T = BS // NT  # 16 tiles per core

F32 = mybir.dt.float32
# bf16 compute: matmul streams 1 col/cycle warm (f32r measures 2), DVE
# tensor_tensor gets 2x mode, DMA bytes halve. rel err ~8e-3 << 2e-2.
DT = mybir.dt.bfloat16
DT_NP = ml_dtypes.bfloat16

AF = mybir.ActivationFunctionType
KCH = ((0, 128), (128, 256), (256, NIA))  # K chunks of the state dim


# ---------------------------------------------------------------- weights
# All stationary operands are packed into one [128, WCOLS] host array;
# each lhsT is a column slice [0:K, off:off+M]. Rows >= K are zero.


class _Pack:
    def __init__(self):
        self.cols = []
        self.off = 0

    def add(self, arr):  # arr [K, M] -> returns (off, K, M)
        k, m = arr.shape
        assert k <= 128
        a = np.zeros((128, m), np.float32)
        a[:k] = arr
        off = self.off
        self.cols.append(a)
        self.off += m
        return (off, k, m)

    def data(self):
        return np.concatenate(self.cols, axis=1)


def _prep_weights(p, bw1, bb1, bw2, bb2, bwo, bbo, ew1, eb1, ew2, eb2, ewm, ebm):
    d = {}
    # blend L1: lhsT [377, 128] (= [bw1.T; bb1]) in 3 K-chunks
    w1a = np.concatenate([bw1.T, bb1[None, :]], axis=0)
    d["bl1"] = [p.add(w1a[k0:k1]) for k0, k1 in KCH]
    d["bl2"] = [p.add(bw2.T)]
    d["bb2"] = p.add(bb2[:, None])
    # router logit diff, replicated to 128 output columns
    wd = (bwo[0] - bwo[1])[:, None]
    d["wd"] = [p.add(np.repeat(wd, 128, axis=1))]
    d["bd"] = p.add(np.full((128, 1), bbo[0] - bbo[1], np.float32))
    # expert L1: base = expert1, diff = expert0 - expert1; bias rows appended
    e1b = np.concatenate([ew1[1].T, eb1[1][None, :]], axis=0)  # [377, 256]
    e1d = np.concatenate([(ew1[0] - ew1[1]).T, (eb1[0] - eb1[1])[None, :]], axis=0)
    d["e1b"] = [[p.add(e1b[k0:k1, m : m + 128]) for k0, k1 in KCH] for m in (0, 128)]
    d["e1d"] = [[p.add(e1d[k0:k1, m : m + 128]) for k0, k1 in KCH] for m in (0, 128)]
    # expert L2
    e2b = ew2[1].T  # [256, 128]
    e2d = (ew2[0] - ew2[1]).T
    d["e2b"] = [p.add(e2b[0:128]), p.add(e2b[128:256])]
    d["e2d"] = [p.add(e2d[0:128]), p.add(e2d[128:256])]
    d["db2"] = [p.add((eb2[0] - eb2[1])[None, :])]  # [1, 128]
    d["b2"] = p.add(eb2[1][:, None])
    # expert out
    d["emb"] = [p.add(ewm[1].T)]  # [128, 17]
    d["emd"] = [p.add((ewm[0] - ewm[1]).T)]
    d["dbm"] = [p.add((ebm[0] - ebm[1])[None, :])]  # [1, 17]
    d["bm"] = p.add(ebm[1][:, None])
    return d


# ---------------------------------------------------------------- kernel


def _build(wd, wcols):
    nc = bacc.Bacc("TRN2", target_bir_lowering=False, debug=False,
                   num_devices=N_CORES)
    xs = nc.declare_dram_parameter("xs", [NIA, BS], DT, isOutput=False)
    wk = nc.declare_dram_parameter("wk", [128, wcols], DT, isOutput=False)
    out = nc.declare_dram_parameter("out", [NA, BS], F32, isOutput=True)

    with tile.TileContext(nc) as tc:
        with (
            tc.tile_pool(name="wpool", bufs=1) as wpool,
            tc.tile_pool(name="spool", bufs=3) as spool,
            tc.tile_pool(name="scpool", bufs=4) as scpool,
            tc.tile_pool(name="gpool", bufs=3) as gpool,
            tc.tile_pool(name="cpool", bufs=4) as cpool,
            tc.tile_pool(name="hpool", bufs=3) as hpool,
            tc.tile_pool(name="opool", bufs=2) as opool,
            tc.tile_pool(name="psum", bufs=1, space="PSUM") as pp,
        ):
            # weight pack arrives as two separate tiles: the blend weights
            # (packed first) unblock the first matmuls early, and the split
            # avoids a false whole-tile dependency on the second DMA
            wsplit = wd["e1b"][0][0][0]  # end of the blend-weight section
            wkt1 = wpool.tile([128, wsplit], DT)
            nc.sync.dma_start(wkt1[:], wk[:, 0:wsplit])
            wkt2 = wpool.tile([128, wcols - wsplit], DT)

            def W(desc):
                off, k, m = desc
                if off + m <= wsplit:
                    return wkt1[0:k, off : off + m]
                return wkt2[0:k, off - wsplit : off - wsplit + m]

            # per-tile (t) and per-pair (p = t//2) live tensors; pairs are
            # 1024 wide so DMA and GpSimd muls run at half the instruction
            # and semaphore count
            s = {}
            sc = {}
            c0 = {}
            g1 = {}
            g2 = {}
            h1 = {}
            h1c = {}
            h2 = {}
            h2c = {}
            mu = {}

            def csl(t):
                return slice(t * NT, (t + 1) * NT)

            def psl(t):  # slice of tile t within its pair tile
                return slice((t % 2) * NT, (t % 2 + 1) * NT)

            def sv(t, ci):  # state chunk view for tile t
                return s[t // 2][ci][:, psl(t)]

            def dma_in(p, chunks=(0, 1, 2)):
                # pair p: columns [p*2NT, (p+1)*2NT); chunks may be staggered
                # across iterations to spread SBUF write-port contention
                if p not in s:
                    s[p] = [None, None, None]
                for ci in chunks:
                    k0, k1 = KCH[ci]
                    st = spool.tile([k1 - k0, 2 * NT], DT, tag=f"s{ci}",
                                    name=f"s{ci}_{p}", bufs=4)
                    nc.sync.dma_start(st[:], xs[k0:k1, p * 2 * NT : (p + 1) * 2 * NT])
                    s[p][ci] = st

            def blend_g1(t):
                pg1 = pp.tile([BH, NT], F32, tag="g1", name=f"pg1_{t}", bufs=2)
                for ci in range(3):
                    nc.tensor.matmul(pg1[:], W(wd["bl1"][ci]), sv(t, ci),
                                     start=(ci == 0), stop=(ci == 2))
                g1[t] = gpool.tile([BH, NT], DT, tag="g1", name=f"g1_{t}")
                nc.vector.tensor_scalar_max(g1[t][:], pg1[:], 0.0)

            def blend_g2(t):
                pg2 = pp.tile([BH, NT], F32, tag="g2", name=f"pg2_{t}")
                nc.tensor.matmul(pg2[:], W(wd["bl2"][0]), g1[t][:],
                                 start=True, stop=True)
                g2[t] = gpool.tile([BH, NT], DT, tag="g2", name=f"g2_{t}")
                nc.scalar.activation(g2[t][:], pg2[:], AF.Relu, bias=W(wd["bb2"]))

            def blend_d(t):
                pd = pp.tile([128, NT], F32, tag="d", name=f"pd_{t}")
                nc.tensor.matmul(pd[:], W(wd["wd"][0]), g2[t][:],
                                 start=True, stop=True)
                p = t // 2
                if t % 2 == 0:
                    c0[p] = cpool.tile([128, 2 * NT], DT, tag="c0", name=f"c0_{p}")
                nc.scalar.activation(c0[p][:, psl(t)], pd[:], AF.Sigmoid,
                                     bias=W(wd["bd"]))
                # scaled states for tile t; chunk 0 on DVE, chunks 1/2 on
                # GpSimd (SBUF-only). Row 120 of chunk 2 becomes c0 via the
                # ones-row of xs.
                sc[t] = []
                for ci, (k0, k1) in enumerate(KCH):
                    tt = scpool.tile([k1 - k0, NT], DT, tag=f"sc{ci}",
                                     name=f"sc{ci}_{t}")
                    eng = nc.vector if ci == 0 else nc.gpsimd
                    eng.tensor_mul(tt[:], sv(t, ci), c0[p][0 : k1 - k0, psl(t)])
                    sc[t].append(tt)

            def exp_l1(t):
                h1[t] = []
                h1c[t] = []
                for m in range(2):
                    ph = pp.tile([128, NT], F32, tag=f"h1{m}", name=f"ph1{m}_{t}")
                    for ci in range(3):
                        nc.tensor.matmul(ph[:], W(wd["e1b"][m][ci]), sv(t, ci),
                                         start=(ci == 0), stop=False)
                    for ci in range(3):
                        nc.tensor.matmul(ph[:], W(wd["e1d"][m][ci]), sc[t][ci][:],
                                         start=False, stop=(ci == 2))
                    ht = hpool.tile([128, NT], DT, tag=f"h1{m}", name=f"h1{m}_{t}")
                    nc.vector.tensor_scalar_max(ht[:], ph[:], 0.0)
                    h1[t].append(ht)
                    htc = hpool.tile([128, NT], DT, tag=f"h1c{m}",
                                     name=f"h1c{m}_{t}")
                    nc.vector.tensor_mul(htc[:], ht[:], c0[t // 2][:, psl(t)])
                    h1c[t].append(htc)

            ph2s = {}
            pmus = {}

            def exp_l2_mm(t):
                ph2s[t] = pp.tile([128, NT], F32, tag="h2", name=f"ph2_{t}")
                ph2 = ph2s[t]
                nc.tensor.matmul(ph2[:], W(wd["e2b"][0]), h1[t][0][:],
                                 start=True, stop=False)
                nc.tensor.matmul(ph2[:], W(wd["e2b"][1]), h1[t][1][:],
                                 start=False, stop=False)
                nc.tensor.matmul(ph2[:], W(wd["e2d"][0]), h1c[t][0][:],
                                 start=False, stop=False)
                nc.tensor.matmul(ph2[:], W(wd["e2d"][1]), h1c[t][1][:],
                                 start=False, stop=False)

            def exp_l3_mm(t):
                pmus[t] = pp.tile([NA, NT], F32, tag="mu", name=f"pmu_{t}")
                pmu = pmus[t]
                nc.tensor.matmul(pmu[:], W(wd["emb"][0]), h2[t][:],
                                 start=True, stop=False)
                nc.tensor.matmul(pmu[:], W(wd["emd"][0]), h2c[t][:],
                                 start=False, stop=False)

            def bias_mms(t2, t3):
                # two K=1 rank-1 bias matmuls; db2 occupies PE row group 0,
                # dbm row group 1 (weights at partition 32), so adjacent
                # instructions stream concurrently through the array
                if t2 is not None:
                    nc.tensor.matmul(ph2s[t2][:], W(wd["db2"][0]),
                                     c0[t2 // 2][0:1, psl(t2)],
                                     start=False, stop=True)
                if t3 is not None:
                    nc.tensor.matmul(pmus[t3][:], W(wd["dbm"][0]),
                                     c0[t3 // 2][0:1, psl(t3)],
                                     start=False, stop=True)

            def exp_l2_post(t):
                ph2 = ph2s.pop(t)
                h2[t] = hpool.tile([128, NT], DT, tag="h2", name=f"h2_{t}")
                nc.scalar.activation(h2[t][:], ph2[:], AF.Relu, bias=W(wd["b2"]))
                h2c[t] = hpool.tile([128, NT], DT, tag="h2c", name=f"h2c_{t}")
                nc.vector.tensor_mul(h2c[t][:], h2[t][:], c0[t // 2][:, psl(t)])
                del g1[t], g2[t], sc[t]
                if t % 2 == 1:
                    del s[t // 2]

            def exp_l3_post(t):
                pmu = pmus.pop(t)
                p = t // 2
                if t % 2 == 0:
                    mu[p] = opool.tile([NA, 2 * NT], F32, tag="mu", name=f"mu_{p}")
                nc.scalar.activation(mu[p][:, psl(t)], pmu[:], AF.Tanh,
                                     bias=W(wd["bm"]))
                if t >= T - 2:
                    nc.sync.dma_start(out[:, csl(t)], mu[p][:, psl(t)])
                    if t % 2 == 1:
                        del mu[p], c0[p]
                elif t % 2 == 1:
                    nc.sync.dma_start(out[:, p * 2 * NT : (p + 1) * 2 * NT],
                                      mu[p][:])
                    del mu[p], c0[p]
                del h1[t], h1c[t], h2[t], h2c[t]

            def exp_l2(t):
                exp_l2_mm(t)
                bias_mms(t, None)
                exp_l2_post(t)

            def exp_l3(t):
                exp_l3_mm(t)
                bias_mms(None, t)
                exp_l3_post(t)

            # -------- software-pipelined emission --------
            # prologue: state pairs 0..1, blend chains for tiles 0 and 1
            # (interleaved to overlap their serial MM->ACT/DVE hops)
            dma_in(0)
            dma_in(1)
            nc.sync.dma_start(wkt2[:], wk[:, wsplit:])
            dma_in(2, (0,))
            blend_g1(0)
            blend_g1(1)
            blend_g2(0)
            blend_g2(1)
            blend_d(0)
            blend_d(1)
            # steady state: iteration t runs L1(t), L2(t-1), L3(t-2) and
            # the blend MLP of t+2 spliced between expert blocks
            for t in range(T):
                if t % 2 == 1 and (t + 5) // 2 < T // 2:
                    dma_in((t + 5) // 2, (0,))
                if t % 2 == 0 and 2 <= (t + 4) // 2 < T // 2:
                    dma_in((t + 4) // 2, (1, 2))
                if t + 2 < T:
                    blend_g1(t + 2)
                exp_l1(t)
                if t + 2 < T:
                    blend_g2(t + 2)
                if t >= 1:
                    exp_l2_mm(t - 1)
                if t >= 2:
                    exp_l3_mm(t - 2)
                bias_mms(t - 1 if t >= 1 else None, t - 2 if t >= 2 else None)
                if t + 2 < T:
                    blend_d(t + 2)
                if t >= 1:
                    exp_l2_post(t - 1)
                if t >= 2:
                    exp_l3_post(t - 2)
            exp_l2(T - 1)
            exp_l3(T - 2)
            exp_l3(T - 1)
    nc.finalize()
    return nc


_CACHE = {}


def kernel(**inputs) -> np.ndarray:
    states = np.asarray(inputs["states"], np.float32)
    pack = _Pack()
    wdesc = _prep_weights(
        pack,
        *[
            np.asarray(inputs[k], np.float32)
            for k in ("bw1", "bb1", "bw2", "bb2", "bwo", "bbo",
                      "ew1", "eb1", "ew2", "eb2", "ewm", "ebm")
        ],
    )
    wdata = pack.data().astype(DT_NP)  # [128, wcols]

    if "nc" not in _CACHE:
        _CACHE["nc"] = _build(wdesc, wdata.shape[1])
    nc = _CACHE["nc"]

    in_maps = []
    for c in range(N_CORES):
        shard = states[c * BS : (c + 1) * BS]  # [BS, NI]
        xs = np.empty((NIA, BS), np.float32)
        xs[:NI] = shard.T
        xs[NI] = 1.0
        in_maps.append({"xs": xs.astype(DT_NP), "wk": wdata})

    res = run_bass_kernel_spmd(nc, in_maps, core_ids=list(range(N_CORES)))
    out = np.empty((B, NA), np.float32)
    for c in range(N_CORES):
        out[c * BS : (c + 1) * BS] = res.results[c]["out"].T
    return out
